# revision 4
# baseline (speedup 1.0000x reference)
"""Single-head self-attention (B=4, T=2048, D=1024) on 8 trn2 NeuronCores.

Sharding: SPMD over (batch, query-half): core c handles batch c//2, query rows
[h*1024, (h+1)*1024) with h = c%2.  Each core computes K/V for its full batch
(duplicated across the 2 cores sharing a batch - no collectives needed).

Device dataflow keeps activations feature-major ([d/e partition, t free]) so
no on-device transposes are needed:
  QT[e,tq] = WqT.T @ xqT          (lhsT=WqT tile, rhs=xqT tile)
  KT[e,tk] = WkT.T @ xkvT          (fused per 512-wide tk block)
  expST[tk,tq] = exp((KT.T @ QT)/32)   (softmax w/o max-subtract: |S|<~3)
  rowsum[1,tq] = ones.T @ expST        (PE column-sum)
  V[tk,e]  = xkvT.T @ WvT
  OT[e,tq] = (V.T @ expST) * (1/rowsum)
  Y[tq,f]  = OT.T @ WoT + bo           (natural layout -> contiguous store)

All matmuls in bf16 (fp32 PSUM accumulation); measured end-to-end error vs
fp32 reference ~2.8e-3 scale-relative absmax.
"""
import numpy as np
import ml_dtypes

import concourse.bacc as bacc
import concourse.tile as tile
from concourse import mybir
from concourse import bass_utils

F32 = mybir.dt.float32
BF16 = mybir.dt.bfloat16
AF = mybir.ActivationFunctionType

N_CORES = 8
P = 128
D = 1024     # input feature dim (8 ptiles)
E = 1024     # projected feature dim (8 ptiles)
TQ = 1024    # query rows per core (8 ptiles)
TK = 2048    # key rows per core (16 ptiles)
DT = D // P  # 8
ET = E // P  # 8
QT_T = TQ // P   # 8
TK_T = TK // P   # 16
BLK = 512        # tk block width for fused KT/ST and V phases
NBLK = TK // BLK  # 4
SUB = BLK // P    # 4

_CACHE = {}


def _build():
    nc = bacc.Bacc("TRN2", target_bir_lowering=False, debug=False,
                   num_devices=N_CORES)
    xq_d = nc.dram_tensor("xqT", [D, TQ], BF16, kind="ExternalInput").ap()
    xkv_d = nc.dram_tensor("xkvT", [D, TK], BF16, kind="ExternalInput").ap()
    wq_d = nc.dram_tensor("wqT", [D, E], BF16, kind="ExternalInput").ap()
    wk_d = nc.dram_tensor("wkT", [D, E], BF16, kind="ExternalInput").ap()
    wv_d = nc.dram_tensor("wvT", [D, E], BF16, kind="ExternalInput").ap()
    wo_d = nc.dram_tensor("woT", [E, D], BF16, kind="ExternalInput").ap()
    bq_d = nc.dram_tensor("bq", [E], F32, kind="ExternalInput").ap()
    bk_d = nc.dram_tensor("bk", [E], F32, kind="ExternalInput").ap()
    bv_d = nc.dram_tensor("bv", [E], F32, kind="ExternalInput").ap()
    bo_d = nc.dram_tensor("bo", [D], F32, kind="ExternalInput").ap()
    y_d = nc.dram_tensor("y", [TQ, D], F32, kind="ExternalOutput").ap()

    with tile.TileContext(nc) as tc:
        import contextlib
        with contextlib.ExitStack() as ctx:
            wpool = ctx.enter_context(tc.tile_pool(name="wpool", bufs=16))
            xqp = ctx.enter_context(tc.tile_pool(name="xqp", bufs=8))
            qtp = ctx.enter_context(tc.tile_pool(name="qtp", bufs=8))
            esp = ctx.enter_context(tc.tile_pool(name="esp", bufs=16))
            vp = ctx.enter_context(tc.tile_pool(name="vp", bufs=16))
            otp = ctx.enter_context(tc.tile_pool(name="otp", bufs=8))
            xslp = ctx.enter_context(tc.tile_pool(name="xslp", bufs=16))
            ktp = ctx.enter_context(tc.tile_pool(name="ktp", bufs=16))
            outp = ctx.enter_context(tc.tile_pool(name="outp", bufs=3))
            cst = ctx.enter_context(tc.tile_pool(name="cst", bufs=1))
            drp = ctx.enter_context(tc.tile_pool(name="drp", bufs=1,
                                                 space="DRAM"))
            ps_big = ctx.enter_context(
                tc.tile_pool(name="ps_big", bufs=2, space="PSUM"))
            ps_k = ctx.enter_context(
                tc.tile_pool(name="ps_k", bufs=2, space="PSUM"))
            ps_rs = ctx.enter_context(
                tc.tile_pool(name="ps_rs", bufs=1, space="PSUM"))

            # ---- constants ----
            bqs = cst.tile([P, ET], F32, tag="bqs", name="bqs")
            nc.sync.dma_start(out=bqs, in_=bq_d.rearrange("(s p) -> p s", p=P))
            bks = cst.tile([P, ET], F32, tag="bks", name="bks")
            nc.sync.dma_start(out=bks, in_=bk_d.rearrange("(s p) -> p s", p=P))
            bvB = cst.tile([P, E], F32, tag="bvB", name="bvB")
            nc.sync.dma_start(out=bvB, in_=bv_d.partition_broadcast(P))
            boB = cst.tile([P, D], F32, tag="boB", name="boB")
            nc.sync.dma_start(out=boB, in_=bo_d.partition_broadcast(P))
            ones = cst.tile([P, 1], BF16, tag="ones", name="ones")
            nc.vector.memset(ones, 1.0)

            # ---- load Wq, Wk (wpool holds 16 slots -> both resident) ----
            wq = [wpool.tile([P, E], BF16, tag="w", name="w") for _ in range(DT)]
            for d in range(DT):
                nc.sync.dma_start(out=wq[d], in_=wq_d[d * P:(d + 1) * P, :])
            wk = [wpool.tile([P, E], BF16, tag="w", name="w") for _ in range(DT)]
            for d in range(DT):
                nc.sync.dma_start(out=wk[d], in_=wk_d[d * P:(d + 1) * P, :])

            # ---- phase 1: QT[e, tq] ----
            xq = [xqp.tile([P, TQ], BF16, tag="xq", name="xq") for _ in range(DT)]
            for d in range(DT):
                nc.sync.dma_start(out=xq[d], in_=xq_d[d * P:(d + 1) * P, :])
            qt = []
            for e in range(ET):
                qps = ps_big.tile([P, TQ], F32, tag="big", name="psb")
                for d in range(DT):
                    lhs = wq[d][:, e * P:(e + 1) * P]
                    nc.tensor.matmul(qps[:, 0:512], lhs, xq[d][:, 0:512],
                                     start=(d == 0), stop=(d == DT - 1))
                    nc.tensor.matmul(qps[:, 512:1024], lhs, xq[d][:, 512:1024],
                                     start=(d == 0), stop=(d == DT - 1))
                t = qtp.tile([P, TQ], BF16, tag="qt", name="qt")
                nc.scalar.activation(out=t, in_=qps, func=AF.Identity,
                                     bias=bqs[:, e:e + 1], scale=1.0)
                qt.append(t)

            # ---- phase 2: fused KT-block -> expST + rowsum ----
            rs_ps = ps_rs.tile([1, TQ], F32, tag="rs", name="psrs")
            es = []
            for blk in range(NBLK):
                xsl = [xslp.tile([P, BLK], BF16, tag="xsl", name="xsl") for _ in range(DT)]
                for d in range(DT):
                    nc.sync.dma_start(
                        out=xsl[d],
                        in_=xkv_d[d * P:(d + 1) * P,
                                  blk * BLK:(blk + 1) * BLK])
                ktb = []
                for e in range(ET):
                    kps = ps_k.tile([P, BLK], F32, tag="k", name="psk")
                    for d in range(DT):
                        nc.tensor.matmul(kps, wk[d][:, e * P:(e + 1) * P],
                                         xsl[d],
                                         start=(d == 0), stop=(d == DT - 1))
                    t = ktp.tile([P, BLK], BF16, tag="ktb", name="ktb")
                    nc.scalar.activation(out=t, in_=kps, func=AF.Identity,
                                         bias=bks[:, e:e + 1], scale=1.0)
                    ktb.append(t)
                for i in range(SUB):
                    tk = blk * SUB + i
                    sps = ps_big.tile([P, TQ], F32, tag="big", name="psb")
                    for e in range(ET):
                        lhs = ktb[e][:, i * P:(i + 1) * P]
                        nc.tensor.matmul(sps[:, 0:512], lhs, qt[e][:, 0:512],
                                         start=(e == 0), stop=(e == ET - 1))
                        nc.tensor.matmul(sps[:, 512:1024], lhs,
                                         qt[e][:, 512:1024],
                                         start=(e == 0), stop=(e == ET - 1))
                    t = esp.tile([P, TQ], BF16, tag="es", name="es")
                    nc.scalar.activation(out=t, in_=sps, func=AF.Exp,
                                         scale=1.0 / 32.0)
                    es.append(t)
                    nc.tensor.matmul(rs_ps[:, 0:512], ones, t[:, 0:512],
                                     start=(tk == 0), stop=(tk == TK_T - 1))
                    nc.tensor.matmul(rs_ps[:, 512:1024], ones, t[:, 512:1024],
                                     start=(tk == 0), stop=(tk == TK_T - 1))

            # ---- rowsum -> reciprocal -> broadcast to [P, TQ] ----
            rs_sb = cst.tile([1, TQ], F32, tag="rs_sb", name="rs_sb")
            nc.vector.reciprocal(out=rs_sb, in_=rs_ps)
            rs_dr = drp.tile([1, TQ], F32, tag="rs_dr", name="rs_dr")
            nc.sync.dma_start(out=rs_dr, in_=rs_sb)
            rb = cst.tile([P, TQ], F32, tag="rb", name="rb")
            nc.sync.dma_start(out=rb, in_=rs_dr[0, :].partition_broadcast(P))

            # ---- phase 3: V[tk, e] (re-reads xkvT) ----
            wv = [wpool.tile([P, E], BF16, tag="w", name="w") for _ in range(DT)]
            for d in range(DT):
                nc.sync.dma_start(out=wv[d], in_=wv_d[d * P:(d + 1) * P, :])
            v = []
            for blk in range(NBLK):
                xsl = [xslp.tile([P, BLK], BF16, tag="xsl", name="xsl") for _ in range(DT)]
                for d in range(DT):
                    nc.sync.dma_start(
                        out=xsl[d],
                        in_=xkv_d[d * P:(d + 1) * P,
                                  blk * BLK:(blk + 1) * BLK])
                for i in range(SUB):
                    vps = ps_big.tile([P, E], F32, tag="big", name="psb")
                    for d in range(DT):
                        lhs = xsl[d][:, i * P:(i + 1) * P]
                        nc.tensor.matmul(vps[:, 0:512], lhs, wv[d][:, 0:512],
                                         start=(d == 0), stop=(d == DT - 1))
                        nc.tensor.matmul(vps[:, 512:1024], lhs,
                                         wv[d][:, 512:1024],
                                         start=(d == 0), stop=(d == DT - 1))
                    t = vp.tile([P, E], BF16, tag="v", name="v")
                    nc.vector.tensor_add(t, vps, bvB)
                    v.append(t)

            # ---- phase 4: OT[e, tq] = (V.T @ expST) * recip_rowsum ----
            wo = [wpool.tile([P, D], BF16, tag="w", name="w") for _ in range(ET)]
            for e in range(ET):
                nc.sync.dma_start(out=wo[e], in_=wo_d[e * P:(e + 1) * P, :])
            ot = []
            for e in range(ET):
                ops_ = ps_big.tile([P, TQ], F32, tag="big", name="psb")
                for tk in range(TK_T):
                    lhs = v[tk][:, e * P:(e + 1) * P]
                    nc.tensor.matmul(ops_[:, 0:512], lhs, es[tk][:, 0:512],
                                     start=(tk == 0), stop=(tk == TK_T - 1))
                    nc.tensor.matmul(ops_[:, 512:1024], lhs,
                                     es[tk][:, 512:1024],
                                     start=(tk == 0), stop=(tk == TK_T - 1))
                t = otp.tile([P, TQ], BF16, tag="ot", name="ot")
                nc.vector.tensor_mul(t, ops_, rb)
                ot.append(t)

            # ---- phase 5: Y[tq, f] = OT.T @ WoT + bo ----
            for q in range(QT_T):
                yps = ps_big.tile([P, D], F32, tag="big", name="psb")
                for e in range(ET):
                    lhs = ot[e][:, q * P:(q + 1) * P]
                    nc.tensor.matmul(yps[:, 0:512], lhs, wo[e][:, 0:512],
                                     start=(e == 0), stop=(e == ET - 1))
                    nc.tensor.matmul(yps[:, 512:1024], lhs, wo[e][:, 512:1024],
                                     start=(e == 0), stop=(e == ET - 1))
                ysb = outp.tile([P, D], F32, tag="y", name="y_sb")
                nc.vector.tensor_add(ysb, yps, boB)
                nc.sync.dma_start(out=y_d[q * P:(q + 1) * P, :], in_=ysb)

    nc.compile()
    return nc


def _get_program():
    if "nc" not in _CACHE:
        _CACHE["nc"] = _build()
    return _CACHE["nc"]


def kernel(x, Wq, bq, Wk, bk, Wv, bv, Wo, bo):
    x = np.asarray(x)
    B, T, _ = x.shape  # (4, 2048, 1024)
    bf = ml_dtypes.bfloat16
    wqT = np.ascontiguousarray(np.asarray(Wq).T).astype(bf)
    wkT = np.ascontiguousarray(np.asarray(Wk).T).astype(bf)
    wvT = np.ascontiguousarray(np.asarray(Wv).T).astype(bf)
    woT = np.ascontiguousarray(np.asarray(Wo).T).astype(bf)
    bq = np.asarray(bq, np.float32)
    bk = np.asarray(bk, np.float32)
    bv = np.asarray(bv, np.float32)
    bo = np.asarray(bo, np.float32)

    in_maps = []
    for c in range(N_CORES):
        b, h = divmod(c, 2)
        xkvT = np.ascontiguousarray(x[b].T).astype(bf)
        xqT = np.ascontiguousarray(x[b, h * TQ:(h + 1) * TQ, :].T).astype(bf)
        in_maps.append({
            "xqT": xqT, "xkvT": xkvT,
            "wqT": wqT, "wkT": wkT, "wvT": wvT, "woT": woT,
            "bq": bq, "bk": bk, "bv": bv, "bo": bo,
        })

    nc = _get_program()
    res = bass_utils.run_bass_kernel_spmd(
        nc, in_maps, core_ids=list(range(N_CORES)))
    out = np.empty((B, T, D), np.float32)
    for c in range(N_CORES):
        b, h = divmod(c, 2)
        out[b, h * TQ:(h + 1) * TQ, :] = res.results[c]["y"]
    return out


# revision 5
# speedup vs baseline: 1.0222x; 1.0222x over previous
"""Single-head self-attention (B=4, T=2048, D=1024) on 8 trn2 NeuronCores.

Sharding: SPMD over (batch, query-half): core c handles batch c//2, query rows
[h*1024, (h+1)*1024) with h = c%2.  Each core computes K/V for its full batch
(duplicated across the 2 cores sharing a batch - no collectives needed).

Device dataflow keeps activations feature-major ([d/e partition, t free]) so
no on-device transposes are needed:
  QT[e,tq] = WqT.T @ xqT          (lhsT=WqT tile, rhs=xqT tile)
  KT[e,tk] = WkT.T @ xkvT          (fused per 512-wide tk block)
  expST[tk,tq] = exp((KT.T @ QT)/32)   (softmax w/o max-subtract: |S|<~3)
  rowsum[1,tq] = ones.T @ expST        (PE column-sum)
  V[tk,e]  = xkvT.T @ WvT
  OT[e,tq] = (V.T @ expST) * (1/rowsum)
  Y[tq,f]  = OT.T @ WoT + bo           (natural layout -> contiguous store)

KT blocks 0-1 are computed first (they only need Wk + a 1MB x slice) so the
PE starts ~10us in, while the larger Wq/xq loads stream behind.

All matmuls in bf16 (fp32 PSUM accumulation); measured end-to-end error vs
fp32 reference ~2.7e-3 scale-relative absmax.
"""
import contextlib

import numpy as np
import ml_dtypes

import concourse.bacc as bacc
import concourse.tile as tile
from concourse import mybir
from concourse import bass_utils

F32 = mybir.dt.float32
BF16 = mybir.dt.bfloat16
AF = mybir.ActivationFunctionType

N_CORES = 8
P = 128
D = 1024     # input feature dim (8 ptiles)
E = 1024     # projected feature dim (8 ptiles)
TQ = 1024    # query rows per core (8 ptiles)
TK = 2048    # key rows per core (16 ptiles)
DT = D // P  # 8
ET = E // P  # 8
QT_T = TQ // P   # 8
TK_T = TK // P   # 16
BLK = 512        # tk block width for fused KT/ST and V phases
NBLK = TK // BLK  # 4
SUB = BLK // P    # 4

_CACHE = {}


def _build():
    nc = bacc.Bacc("TRN2", target_bir_lowering=False, debug=False,
                   num_devices=N_CORES)
    xq_d = nc.dram_tensor("xqT", [D, TQ], BF16, kind="ExternalInput").ap()
    xkv_d = nc.dram_tensor("xkvT", [D, TK], BF16, kind="ExternalInput").ap()
    wq_d = nc.dram_tensor("wqT", [D, E], BF16, kind="ExternalInput").ap()
    wk_d = nc.dram_tensor("wkT", [D, E], BF16, kind="ExternalInput").ap()
    wv_d = nc.dram_tensor("wvT", [D, E], BF16, kind="ExternalInput").ap()
    wo_d = nc.dram_tensor("woT", [E, D], BF16, kind="ExternalInput").ap()
    bq_d = nc.dram_tensor("bq", [E], F32, kind="ExternalInput").ap()
    bk_d = nc.dram_tensor("bk", [E], F32, kind="ExternalInput").ap()
    bv_d = nc.dram_tensor("bv", [E], F32, kind="ExternalInput").ap()
    bo_d = nc.dram_tensor("bo", [D], F32, kind="ExternalInput").ap()
    y_d = nc.dram_tensor("y", [TQ, D], F32, kind="ExternalOutput").ap()

    with tile.TileContext(nc) as tc:
        with contextlib.ExitStack() as ctx:
            wpool = ctx.enter_context(tc.tile_pool(name="wpool", bufs=16))
            xqp = ctx.enter_context(tc.tile_pool(name="xqp", bufs=8))
            qtp = ctx.enter_context(tc.tile_pool(name="qtp", bufs=8))
            esp = ctx.enter_context(tc.tile_pool(name="esp", bufs=16))
            vp = ctx.enter_context(tc.tile_pool(name="vp", bufs=16))
            otp = ctx.enter_context(tc.tile_pool(name="otp", bufs=8))
            xslp = ctx.enter_context(tc.tile_pool(name="xslp", bufs=16))
            ktp = ctx.enter_context(tc.tile_pool(name="ktp", bufs=16))
            outp = ctx.enter_context(tc.tile_pool(name="outp", bufs=3))
            cst = ctx.enter_context(tc.tile_pool(name="cst", bufs=1))
            drp = ctx.enter_context(tc.tile_pool(name="drp", bufs=1,
                                                 space="DRAM"))
            ps_big = ctx.enter_context(
                tc.tile_pool(name="ps_big", bufs=2, space="PSUM"))
            ps_k = ctx.enter_context(
                tc.tile_pool(name="ps_k", bufs=2, space="PSUM"))
            ps_rs = ctx.enter_context(
                tc.tile_pool(name="ps_rs", bufs=1, space="PSUM"))

            # ---- constants (tiny DMAs first) ----
            bqs = cst.tile([P, ET], F32, tag="bqs", name="bqs")
            nc.sync.dma_start(out=bqs, in_=bq_d.rearrange("(s p) -> p s", p=P))
            bks = cst.tile([P, ET], F32, tag="bks", name="bks")
            nc.sync.dma_start(out=bks, in_=bk_d.rearrange("(s p) -> p s", p=P))
            bvB = cst.tile([P, E], F32, tag="bvB", name="bvB")
            nc.sync.dma_start(out=bvB, in_=bv_d.partition_broadcast(P))
            boB = cst.tile([P, D], F32, tag="boB", name="boB")
            nc.sync.dma_start(out=boB, in_=bo_d.partition_broadcast(P))
            ones = cst.tile([P, 1], BF16, tag="ones", name="ones")
            nc.vector.memset(ones, 1.0)

            def load_xsl(blk):
                xsl = [xslp.tile([P, BLK], BF16, tag="xsl", name="xsl")
                       for _ in range(DT)]
                for d in range(DT):
                    nc.sync.dma_start(
                        out=xsl[d],
                        in_=xkv_d[d * P:(d + 1) * P,
                                  blk * BLK:(blk + 1) * BLK])
                return xsl

            # ---- Wk + x slices for KT blocks 0/1 (interleaved so the first
            # KT accumulation can start as soon as possible) ----
            wk = [wpool.tile([P, E], BF16, tag="w", name="w")
                  for _ in range(DT)]
            xsl0 = [xslp.tile([P, BLK], BF16, tag="xsl", name="xsl")
                    for _ in range(DT)]
            for d in range(DT):
                nc.sync.dma_start(out=wk[d], in_=wk_d[d * P:(d + 1) * P, :])
                nc.sync.dma_start(
                    out=xsl0[d], in_=xkv_d[d * P:(d + 1) * P, 0:BLK])
            xsl1 = load_xsl(1)
            xsl_of = {0: xsl0, 1: xsl1}

            # ---- Wq/xq stream behind the KT-block compute ----
            wq = [wpool.tile([P, E], BF16, tag="w", name="w")
                  for _ in range(DT)]
            for d in range(DT):
                nc.sync.dma_start(out=wq[d], in_=wq_d[d * P:(d + 1) * P, :])
            xq = [xqp.tile([P, TQ], BF16, tag="xq", name="xq")
                  for _ in range(DT)]
            for d in range(DT):
                nc.sync.dma_start(out=xq[d], in_=xq_d[d * P:(d + 1) * P, :])

            def kt_block(blk):
                xsl = xsl_of[blk]
                ktb = []
                for e in range(ET):
                    kps = ps_k.tile([P, BLK], F32, tag="k", name="psk")
                    for d in range(DT):
                        nc.tensor.matmul(kps, wk[d][:, e * P:(e + 1) * P],
                                         xsl[d],
                                         start=(d == 0), stop=(d == DT - 1))
                    t = ktp.tile([P, BLK], BF16, tag="ktb", name="ktb")
                    nc.scalar.activation(out=t, in_=kps, func=AF.Identity,
                                         bias=bks[:, e:e + 1], scale=1.0)
                    ktb.append(t)
                return ktb

            ktb_of = {0: kt_block(0), 1: kt_block(1)}

            # ---- QT[e, tq] ----
            qt = []
            for e in range(ET):
                qps = ps_big.tile([P, TQ], F32, tag="big", name="psb")
                for d in range(DT):
                    lhs = wq[d][:, e * P:(e + 1) * P]
                    nc.tensor.matmul(qps[:, 0:512], lhs, xq[d][:, 0:512],
                                     start=(d == 0), stop=(d == DT - 1))
                    nc.tensor.matmul(qps[:, 512:1024], lhs, xq[d][:, 512:1024],
                                     start=(d == 0), stop=(d == DT - 1))
                t = qtp.tile([P, TQ], BF16, tag="qt", name="qt")
                nc.scalar.activation(out=t, in_=qps, func=AF.Identity,
                                     bias=bqs[:, e:e + 1], scale=1.0)
                qt.append(t)

            # ---- expST blocks + rowsum ----
            rs_ps = ps_rs.tile([1, TQ], F32, tag="rs", name="psrs")
            es = []

            def st_block(blk):
                ktb = ktb_of.pop(blk)
                for i in range(SUB):
                    tk = blk * SUB + i
                    sps = ps_big.tile([P, TQ], F32, tag="big", name="psb")
                    for e in range(ET):
                        lhs = ktb[e][:, i * P:(i + 1) * P]
                        nc.tensor.matmul(sps[:, 0:512], lhs, qt[e][:, 0:512],
                                         start=(e == 0), stop=(e == ET - 1))
                        nc.tensor.matmul(sps[:, 512:1024], lhs,
                                         qt[e][:, 512:1024],
                                         start=(e == 0), stop=(e == ET - 1))
                    t = esp.tile([P, TQ], BF16, tag="es", name="es")
                    nc.scalar.activation(out=t, in_=sps, func=AF.Exp,
                                         scale=1.0 / 32.0)
                    es.append(t)
                    nc.tensor.matmul(rs_ps[:, 0:512], ones, t[:, 0:512],
                                     start=(tk == 0), stop=(tk == TK_T - 1))
                    nc.tensor.matmul(rs_ps[:, 512:1024], ones, t[:, 512:1024],
                                     start=(tk == 0), stop=(tk == TK_T - 1))

            st_block(0)
            st_block(1)
            for blk in (2, 3):
                xsl_of[blk] = load_xsl(blk)
                ktb_of[blk] = kt_block(blk)
                st_block(blk)

            # ---- rowsum -> reciprocal -> broadcast to [P, TQ] ----
            rs_sb = cst.tile([1, TQ], F32, tag="rs_sb", name="rs_sb")
            nc.vector.reciprocal(out=rs_sb, in_=rs_ps)
            rs_dr = drp.tile([1, TQ], F32, tag="rs_dr", name="rs_dr")
            nc.sync.dma_start(out=rs_dr, in_=rs_sb)
            rb = cst.tile([P, TQ], F32, tag="rb", name="rb")
            nc.sync.dma_start(out=rb, in_=rs_dr[0, :].partition_broadcast(P))

            # ---- V[tk, e] (re-reads xkvT) ----
            wv = [wpool.tile([P, E], BF16, tag="w", name="w")
                  for _ in range(DT)]
            for d in range(DT):
                nc.sync.dma_start(out=wv[d], in_=wv_d[d * P:(d + 1) * P, :])
            v = []
            for blk in range(NBLK):
                xsl = load_xsl(blk)
                for i in range(SUB):
                    vps = ps_big.tile([P, E], F32, tag="big", name="psb")
                    for d in range(DT):
                        lhs = xsl[d][:, i * P:(i + 1) * P]
                        nc.tensor.matmul(vps[:, 0:512], lhs, wv[d][:, 0:512],
                                         start=(d == 0), stop=(d == DT - 1))
                        nc.tensor.matmul(vps[:, 512:1024], lhs,
                                         wv[d][:, 512:1024],
                                         start=(d == 0), stop=(d == DT - 1))
                    t = vp.tile([P, E], BF16, tag="v", name="v")
                    nc.vector.tensor_add(t, vps, bvB)
                    v.append(t)

            # ---- OT[e, tq] = (V.T @ expST) * recip_rowsum ----
            wo = [wpool.tile([P, D], BF16, tag="w", name="w")
                  for _ in range(ET)]
            for e in range(ET):
                nc.sync.dma_start(out=wo[e], in_=wo_d[e * P:(e + 1) * P, :])
            ot = []
            for e in range(ET):
                ops_ = ps_big.tile([P, TQ], F32, tag="big", name="psb")
                for tk in range(TK_T):
                    lhs = v[tk][:, e * P:(e + 1) * P]
                    nc.tensor.matmul(ops_[:, 0:512], lhs, es[tk][:, 0:512],
                                     start=(tk == 0), stop=(tk == TK_T - 1))
                    nc.tensor.matmul(ops_[:, 512:1024], lhs,
                                     es[tk][:, 512:1024],
                                     start=(tk == 0), stop=(tk == TK_T - 1))
                t = otp.tile([P, TQ], BF16, tag="ot", name="ot")
                nc.vector.tensor_mul(t, ops_, rb)
                ot.append(t)

            # ---- Y[tq, f] = OT.T @ WoT + bo ----
            for q in range(QT_T):
                yps = ps_big.tile([P, D], F32, tag="big", name="psb")
                for e in range(ET):
                    lhs = ot[e][:, q * P:(q + 1) * P]
                    nc.tensor.matmul(yps[:, 0:512], lhs, wo[e][:, 0:512],
                                     start=(e == 0), stop=(e == ET - 1))
                    nc.tensor.matmul(yps[:, 512:1024], lhs, wo[e][:, 512:1024],
                                     start=(e == 0), stop=(e == ET - 1))
                ysb = outp.tile([P, D], F32, tag="y", name="y_sb")
                nc.vector.tensor_add(ysb, yps, boB)
                nc.sync.dma_start(out=y_d[q * P:(q + 1) * P, :], in_=ysb)

    nc.compile()
    return nc


def _get_program():
    if "nc" not in _CACHE:
        _CACHE["nc"] = _build()
    return _CACHE["nc"]


def _make_in_maps(x, Wq, bq, Wk, bk, Wv, bv, Wo, bo):
    bf = ml_dtypes.bfloat16
    wqT = np.ascontiguousarray(np.asarray(Wq).T).astype(bf)
    wkT = np.ascontiguousarray(np.asarray(Wk).T).astype(bf)
    wvT = np.ascontiguousarray(np.asarray(Wv).T).astype(bf)
    woT = np.ascontiguousarray(np.asarray(Wo).T).astype(bf)
    bq = np.asarray(bq, np.float32)
    bk = np.asarray(bk, np.float32)
    bv = np.asarray(bv, np.float32)
    bo = np.asarray(bo, np.float32)
    in_maps = []
    for c in range(N_CORES):
        b, h = divmod(c, 2)
        xkvT = np.ascontiguousarray(x[b].T).astype(bf)
        xqT = np.ascontiguousarray(x[b, h * TQ:(h + 1) * TQ, :].T).astype(bf)
        in_maps.append({
            "xqT": xqT, "xkvT": xkvT,
            "wqT": wqT, "wkT": wkT, "wvT": wvT, "woT": woT,
            "bq": bq, "bk": bk, "bv": bv, "bo": bo,
        })
    return in_maps


def kernel(x, Wq, bq, Wk, bk, Wv, bv, Wo, bo):
    x = np.asarray(x)
    B, T, _ = x.shape  # (4, 2048, 1024)
    in_maps = _make_in_maps(x, Wq, bq, Wk, bk, Wv, bv, Wo, bo)
    nc = _get_program()
    res = bass_utils.run_bass_kernel_spmd(
        nc, in_maps, core_ids=list(range(N_CORES)))
    out = np.empty((B, T, D), np.float32)
    for c in range(N_CORES):
        b, h = divmod(c, 2)
        out[b, h * TQ:(h + 1) * TQ, :] = res.results[c]["y"]
    return out
